# revision 3
# baseline (speedup 1.0000x reference)
"""EdgeConv2dDiff Trainium2 Bass kernel.

Reference computation (B=1, C=64, N=50000, K=16, COUT=64):
    e = concat([x_i, x_j - x_i], axis=channel)          # [B, 2C, N, K]
    y = relu(einsum("bcnk,oc->bonk", e, W) + b)          # [B, COUT, N, K]
    out = max(y, axis=K, keepdims=True)                  # [B, COUT, N, 1]

Algebraic restructuring used here:
    W1 @ x_i + W2 @ (x_j - x_i) == (W1 - W2) @ x_i + W2 @ x_j
so the folded weight  wT = [[(W1-W2).T], [W2.T]]  ([2C, COUT]) turns the
whole edge-feature construction into a single 128-contraction matmul over
a stacked input [x_i; x_j] ([2C, N*K]).  Also
    max_k(relu(z_k + b)) == relu(max_k(z_k) + b)
so the K-max runs on raw PSUM output and bias+relu touches 16x fewer
elements.

Sharding: data-parallel over nodes N across 8 cores (6250 nodes each),
no cross-core communication.
"""

import sys

import numpy as np

for _p in ("/opt/trn_rl_repo",):
    if _p not in sys.path:
        sys.path.insert(0, _p)

B, C, N, K = 1, 64, 50000, 16
COUT = 64
NCORES = 8
NS = N // NCORES          # 6250 nodes per core
FS = NS * K               # 100000 matmul columns per core
CHUNK_NODES = 512         # nodes per DMA chunk ([128, 8192] f32 = 4MB)
MM_NODES = 32             # nodes per matmul (32*16 = 512 = max fp32 free dim)

_CACHE = {}


def _build():
    if "nc" in _CACHE:
        return _CACHE["nc"]
    import concourse.bacc as bacc
    import concourse.mybir as mybir
    from concourse.tile import TileContext

    fp32 = mybir.dt.float32
    nc = bacc.Bacc(
        "TRN2", target_bir_lowering=False, debug=False, num_devices=NCORES
    )
    x = nc.dram_tensor("x", [2 * C, FS], fp32, kind="ExternalInput")
    wT = nc.dram_tensor("wT", [2 * C, COUT], fp32, kind="ExternalInput")
    bias = nc.dram_tensor("bias", [COUT, 1], fp32, kind="ExternalInput")
    y = nc.dram_tensor("y", [COUT, NS], fp32, kind="ExternalOutput")

    with TileContext(nc) as tc:
        with (
            tc.tile_pool(name="const", bufs=1) as cpool,
            tc.tile_pool(name="xin", bufs=3) as xpool,
            tc.tile_pool(name="psum", bufs=8, space="PSUM") as ppool,
            tc.tile_pool(name="out", bufs=3) as opool,
        ):
            wt = cpool.tile([2 * C, COUT], fp32)
            nc.sync.dma_start(wt[:], wT[:])
            bt = cpool.tile([COUT, 1], fp32)
            nc.sync.dma_start(bt[:], bias[:])

            node = 0
            while node < NS:
                nn_ = min(CHUNK_NODES, NS - node)
                cols = nn_ * K
                xt = xpool.tile([2 * C, CHUNK_NODES * K], fp32, tag="xt")
                nc.sync.dma_start(xt[:, :cols], x[:, node * K : node * K + cols])
                ot = opool.tile([COUT, CHUNK_NODES], fp32, tag="ot")
                sub = 0
                while sub < nn_:
                    sn = min(MM_NODES, nn_ - sub)
                    ps = ppool.tile([COUT, MM_NODES * K], fp32, tag="ps")
                    nc.tensor.matmul(
                        ps[:, : sn * K],
                        wt[:],
                        xt[:, sub * K : (sub + sn) * K],
                        start=True,
                        stop=True,
                    )
                    nc.vector.tensor_reduce(
                        ot[:, sub : sub + sn],
                        ps[:, : sn * K].rearrange("p (n k) -> p n k", k=K),
                        axis=mybir.AxisListType.X,
                        op=mybir.AluOpType.max,
                    )
                    sub += sn
                nc.scalar.activation(
                    ot[:, :nn_],
                    ot[:, :nn_],
                    mybir.ActivationFunctionType.Relu,
                    bias=bt[:],
                    scale=1.0,
                )
                nc.sync.dma_start(y[:, node : node + nn_], ot[:, :nn_])
                node += nn_

    nc.compile()
    _CACHE["nc"] = nc
    return nc


def _prep_inputs(x_i, x_j, W, b):
    x_i = np.asarray(x_i, dtype=np.float32).reshape(C, N * K)
    x_j = np.asarray(x_j, dtype=np.float32).reshape(C, N * K)
    W = np.asarray(W, dtype=np.float32)
    b = np.asarray(b, dtype=np.float32)

    W1, W2 = W[:, :C], W[:, C:]
    wT = np.ascontiguousarray(
        np.concatenate([(W1 - W2).T, W2.T], axis=0)
    )  # [2C, COUT]
    bias = np.ascontiguousarray(b.reshape(COUT, 1))

    xfull = np.empty((NCORES, 2 * C, FS), dtype=np.float32)
    for s in range(NCORES):
        xfull[s, :C] = x_i[:, s * FS : (s + 1) * FS]
        xfull[s, C:] = x_j[:, s * FS : (s + 1) * FS]

    return [
        {"x": xfull[s], "wT": wT, "bias": bias} for s in range(NCORES)
    ]


def run(x_i, x_j, W, b, **spmd_kwargs):
    """Build + run, returning (full_output, BassKernelResults)."""
    from concourse.bass_utils import run_bass_kernel_spmd

    nc = _build()
    in_maps = _prep_inputs(x_i, x_j, W, b)
    res = run_bass_kernel_spmd(nc, in_maps, list(range(NCORES)), **spmd_kwargs)
    y = np.concatenate(
        [res.results[s]["y"] for s in range(NCORES)], axis=1
    )  # [COUT, N]
    return y.reshape(B, COUT, N, 1), res


def kernel(x_i, x_j, W, b):
    out, _ = run(x_i, x_j, W, b)
    return out


# revision 6
# speedup vs baseline: 1.1270x; 1.1270x over previous
"""EdgeConv2dDiff Trainium2 Bass kernel.

Reference computation (B=1, C=64, N=50000, K=16, COUT=64):
    e = concat([x_i, x_j - x_i], axis=channel)          # [B, 2C, N, K]
    y = relu(einsum("bcnk,oc->bonk", e, W) + b)          # [B, COUT, N, K]
    out = max(y, axis=K, keepdims=True)                  # [B, COUT, N, 1]

Algebraic restructuring used here:
    W1 @ x_i + W2 @ (x_j - x_i) == (W1 - W2) @ x_i + W2 @ x_j
so the folded weight  wT = [[(W1-W2).T], [W2.T]]  ([2C, COUT]) turns the
whole edge-feature construction into a single 128-contraction matmul over
a stacked input [x_i; x_j] ([2C, N*K]).  Also
    max_k(relu(z_k + b)) == relu(max_k(z_k) + b)
so the K-max runs on raw PSUM output and bias+relu touches 16x fewer
elements.

Sharding: data-parallel over nodes N across 8 cores (6250 nodes each),
no cross-core communication.
"""

import sys

import numpy as np

for _p in ("/opt/trn_rl_repo",):
    if _p not in sys.path:
        sys.path.insert(0, _p)

B, C, N, K = 1, 64, 50000, 16
COUT = 64
NCORES = 8
NS = N // NCORES          # 6250 nodes per core
FS = NS * K               # 100000 matmul columns per core
CHUNK_NODES = 512         # nodes per DMA chunk ([128, 8192] f32 = 4MB)
MM_NODES = 32             # nodes per matmul (32*16 = 512 = max fp32 free dim)

_CACHE = {}


def _build():
    if "nc" in _CACHE:
        return _CACHE["nc"]
    import concourse.bacc as bacc
    import concourse.mybir as mybir
    from concourse.tile import TileContext

    fp32 = mybir.dt.float32
    nc = bacc.Bacc(
        "TRN2", target_bir_lowering=False, debug=False, num_devices=NCORES
    )
    x = nc.dram_tensor("x", [2 * C, FS], fp32, kind="ExternalInput")
    wT = nc.dram_tensor("wT", [2 * C, COUT], fp32, kind="ExternalInput")
    bias = nc.dram_tensor("bias", [2 * C, 1], fp32, kind="ExternalInput")
    y = nc.dram_tensor("y", [COUT, NS], fp32, kind="ExternalOutput")

    with TileContext(nc) as tc:
        with (
            tc.tile_pool(name="const", bufs=1) as cpool,
            tc.tile_pool(name="xin", bufs=5) as xpool,
            tc.tile_pool(name="psum", bufs=8, space="PSUM") as ppool,
            tc.tile_pool(name="out", bufs=3) as opool,
        ):
            wt = cpool.tile([2 * C, COUT], fp32)
            nc.sync.dma_start(wt[:], wT[:])
            # bias replicated onto both partition halves ([2C, 1])
            bt = cpool.tile([2 * C, 1], fp32)
            nc.sync.dma_start(bt[:], bias[:])

            node = 0
            while node < NS:
                nn_ = min(CHUNK_NODES, NS - node)
                cols = nn_ * K
                xt = xpool.tile([2 * C, CHUNK_NODES * K], fp32, tag="xt")
                nc.sync.dma_start(xt[:, :cols], x[:, node * K : node * K + cols])
                ot = opool.tile([2 * C, CHUNK_NODES // 2], fp32, tag="ot")
                # Split the chunk's nodes into two contiguous halves; pair
                # group t of half A (-> psum partitions 0:64) with group t
                # of half B (-> partitions 64:128) so each vector reduce
                # covers all 128 partitions.
                npairs = (nn_ + 2 * MM_NODES - 1) // (2 * MM_NODES)
                ha = min(MM_NODES * npairs, nn_)  # half A node count
                for t in range(npairs):
                    a0 = t * MM_NODES
                    an = min(MM_NODES, ha - a0)
                    b0 = ha + t * MM_NODES
                    bn = max(0, min(MM_NODES, nn_ - b0))
                    ps = ppool.tile([2 * C, MM_NODES * K], fp32, tag="ps")
                    nc.tensor.matmul(
                        ps[0:COUT, : an * K],
                        wt[:],
                        xt[:, a0 * K : (a0 + an) * K],
                        start=True,
                        stop=True,
                    )
                    if bn > 0:
                        nc.tensor.matmul(
                            ps[COUT : 2 * COUT, : bn * K],
                            wt[:],
                            xt[:, b0 * K : (b0 + bn) * K],
                            start=True,
                            stop=True,
                        )
                    if an == bn:
                        nc.vector.tensor_reduce(
                            ot[:, a0 : a0 + an],
                            ps[:, : an * K].rearrange("p (n k) -> p n k", k=K),
                            axis=mybir.AxisListType.X,
                            op=mybir.AluOpType.max,
                        )
                    else:  # ragged tail pair: reduce halves separately
                        nc.vector.tensor_reduce(
                            ot[0:COUT, a0 : a0 + an],
                            ps[0:COUT, : an * K].rearrange(
                                "p (n k) -> p n k", k=K
                            ),
                            axis=mybir.AxisListType.X,
                            op=mybir.AluOpType.max,
                        )
                        if bn > 0:
                            nc.vector.tensor_reduce(
                                ot[COUT : 2 * COUT, a0 : a0 + bn],
                                ps[COUT : 2 * COUT, : bn * K].rearrange(
                                    "p (n k) -> p n k", k=K
                                ),
                                axis=mybir.AxisListType.X,
                                op=mybir.AluOpType.max,
                            )
                hb = nn_ - ha  # half B node count
                nc.scalar.activation(
                    ot[:, :ha],
                    ot[:, :ha],
                    mybir.ActivationFunctionType.Relu,
                    bias=bt[:],
                    scale=1.0,
                )
                nc.sync.dma_start(y[:, node : node + ha], ot[0:COUT, :ha])
                if hb > 0:
                    nc.sync.dma_start(
                        y[:, node + ha : node + nn_],
                        ot[COUT : 2 * COUT, :hb],
                    )
                node += nn_

    nc.compile()
    _CACHE["nc"] = nc
    return nc


def _prep_inputs(x_i, x_j, W, b):
    x_i = np.asarray(x_i, dtype=np.float32).reshape(C, N * K)
    x_j = np.asarray(x_j, dtype=np.float32).reshape(C, N * K)
    W = np.asarray(W, dtype=np.float32)
    b = np.asarray(b, dtype=np.float32)

    W1, W2 = W[:, :C], W[:, C:]
    wT = np.ascontiguousarray(
        np.concatenate([(W1 - W2).T, W2.T], axis=0)
    )  # [2C, COUT]
    bias = np.ascontiguousarray(
        np.concatenate([b, b]).reshape(2 * C, 1)
    )  # replicated onto both partition halves

    xfull = np.empty((NCORES, 2 * C, FS), dtype=np.float32)
    for s in range(NCORES):
        xfull[s, :C] = x_i[:, s * FS : (s + 1) * FS]
        xfull[s, C:] = x_j[:, s * FS : (s + 1) * FS]

    return [
        {"x": xfull[s], "wT": wT, "bias": bias} for s in range(NCORES)
    ]


def run(x_i, x_j, W, b, **spmd_kwargs):
    """Build + run, returning (full_output, BassKernelResults)."""
    from concourse.bass_utils import run_bass_kernel_spmd

    nc = _build()
    in_maps = _prep_inputs(x_i, x_j, W, b)
    res = run_bass_kernel_spmd(nc, in_maps, list(range(NCORES)), **spmd_kwargs)
    y = np.concatenate(
        [res.results[s]["y"] for s in range(NCORES)], axis=1
    )  # [COUT, N]
    return y.reshape(B, COUT, N, 1), res


def kernel(x_i, x_j, W, b):
    out, _ = run(x_i, x_j, W, b)
    return out
